# revision 6
# baseline (speedup 1.0000x reference)
"""MACE GNN layer on 8 Trainium2 NeuronCores (Bass/Tile).

Strategy
--------
- Nodes sharded by receiver-core (node n -> core n // (N/8)); within each core
  nodes are re-ordered grouped by species (fixed-size padded segments) so the
  per-species selector / symmetric-contraction become segment matmuls.
- Edges sharded by receiver core and sorted into 128-node receiver windows so
  the scatter-add becomes PSUM-accumulated one-hot matmuls on the PE.
- The Y1 (unit edge vector) mixing of the tensor-product messages is folded
  into *weighted* one-hot scatter matrices (coefficients 1, Y1_x, Y1_y, Y1_z),
  so the DVE only computes the 11 elementwise radial-weight products.
- The up-projected node feature table (bf16, [ncore*NLP, 256] rows
  [s | v_x | v_y | v_z]) is built per-core and AllGathered through HBM; sender
  features are fetched with SWDGE dma_gather (512B rows).
"""

import math
import numpy as np
import ml_dtypes

import concourse.bacc as bacc
import concourse.bass as bass
import concourse.mybir as mybir
from concourse import tile
from concourse.bass_utils import run_bass_kernel_spmd

F32 = mybir.dt.float32
BF16 = mybir.dt.bfloat16
I16 = mybir.dt.int16
BFNP = ml_dtypes.bfloat16

CORES = 8
C = 64          # channels
S = 10          # species
RB = 8          # radial basis
RH = 64         # radial hidden
NPATH = 5
AVG_NEIGH = 16.0
G = 8           # chunks (128 edges each) per supertile
AF = mybir.ActivationFunctionType
OP = mybir.AluOpType


# --------------------------------------------------------------------------
# host-side preparation
# --------------------------------------------------------------------------

def _prep(vectors, node_s, node_v, radial_embedding, node_specie, senders,
          receivers):
    N = node_s.shape[0]
    E = vectors.shape[0]
    assert N % CORES == 0
    NLOC = N // CORES

    vectors = np.asarray(vectors, np.float32)
    node_s = np.asarray(node_s, np.float32)
    node_v = np.asarray(node_v, np.float32)
    radial_embedding = np.asarray(radial_embedding, np.float32)
    specie = np.asarray(node_specie).astype(np.int64)
    senders = np.asarray(senders).astype(np.int64)
    receivers = np.asarray(receivers).astype(np.int64)

    core_of = np.arange(N) // NLOC
    cnt = np.zeros((CORES, S), np.int64)
    for k in range(CORES):
        cnt[k] = np.bincount(specie[k * NLOC:(k + 1) * NLOC], minlength=S)
    SEGP = int(math.ceil(cnt.max() / 64.0)) * 64
    NLP = S * SEGP                      # padded local node count
    assert NLP % 128 == 0
    NWIN = NLP // 128
    assert CORES * NLP < 32768, "table rows must fit int16"

    loc_pos = np.zeros(N, np.int64)
    for k in range(CORES):
        sl = slice(k * NLOC, (k + 1) * NLOC)
        sp = specie[sl]
        order = np.argsort(sp, kind="stable")
        ranks = np.empty(NLOC, np.int64)
        seg_off = np.zeros(S, np.int64)
        seg_off[1:] = np.cumsum(cnt[k])[:-1]
        ranks[order] = np.arange(NLOC) - seg_off[sp[order]]
        loc_pos[sl] = sp * SEGP + ranks
    grow = core_of * NLP + loc_pos      # global table row

    # ---- edge bucketing by (receiver core, 128-node window) ----
    ek = receivers // NLOC
    loc_r = loc_pos[receivers]
    win_e = loc_r // 128
    col_e = loc_r % 128

    cnt_kw = np.zeros((CORES, NWIN), np.int64)
    for k in range(CORES):
        cnt_kw[k] = np.bincount(win_e[ek == k], minlength=NWIN)
    M_w = np.ceil(cnt_kw.max(axis=0) / 128.0).astype(np.int64)
    CT = int(M_w.sum())
    pad_ch = (G - CT % G) % G
    if pad_ch:
        nz = np.nonzero(M_w)[0]
        M_w[nz[-1] if len(nz) else NWIN - 1] += pad_ch
        CT += pad_ch
    NST = CT // G
    EPAD = CT * 128

    chunk_win = np.repeat(np.arange(NWIN), M_w)      # [CT]
    win_first = np.zeros(CT, bool)
    win_last = np.zeros(CT, bool)
    off = 0
    for w in range(NWIN):
        if M_w[w]:
            win_first[off] = True
            win_last[off + M_w[w] - 1] = True
            off += M_w[w]

    win_ch_off = np.zeros(NWIN + 1, np.int64)
    win_ch_off[1:] = np.cumsum(M_w)

    per_core = []
    for k in range(CORES):
        idx = np.nonzero(ek == k)[0]
        w = win_e[idx]
        order = np.argsort(w, kind="stable")
        idx = idx[order]
        w = w[order]
        wc = np.bincount(w, minlength=NWIN)
        starts = np.concatenate([[0], np.cumsum(wc)[:-1]])
        rank_in_w = np.arange(len(idx)) - np.repeat(starts, wc)
        slot = win_ch_off[w] * 128 + rank_in_w
        slot_edge = np.full(EPAD, -1, np.int64)
        slot_edge[slot] = idx

        mask = slot_edge >= 0
        se = np.where(mask, slot_edge, 0)
        vec_f = np.where(mask[:, None], vectors[se], 0.0).astype(np.float32)
        rad_f = np.where(mask[:, None], radial_embedding[se], 0.0).astype(np.float32)
        oh_f = np.where(mask, col_e[se].astype(np.float32), -1.0).astype(np.float32)
        gx_f = np.where(mask, grow[senders[se]], 0).astype(np.int16)

        vec_st = vec_f.reshape(NST, G, 128, 3).transpose(0, 2, 1, 3).reshape(
            NST, 128, G * 3).copy()
        rad_st = rad_f.reshape(NST, G * 128, RB).transpose(0, 2, 1).copy()
        oh_st = oh_f.reshape(NST, G, 128).transpose(0, 2, 1).astype(BFNP).copy()
        gx_st = np.tile(gx_f.reshape(NST, 64, 16).transpose(0, 2, 1),
                        (1, 8, 1)).copy()

        nsT = np.zeros((C, NLP), np.float32)
        nvT = np.zeros((C, 3 * NLP), np.float32)
        sl = slice(k * NLOC, (k + 1) * NLOC)
        lp = loc_pos[sl]
        nsT[:, lp] = node_s[sl].T
        for d in range(3):
            nvT[:, d * NLP + lp] = node_v[sl, :, d].T
        per_core.append(dict(vec=vec_st, rad=rad_st, oh=oh_st, gx=gx_st,
                             nsT=nsT, nvT=nvT))

    meta = dict(N=N, E=E, NLOC=NLOC, SEGP=SEGP, NLP=NLP, NWIN=NWIN,
                NST=NST, CT=CT,
                chunk_win=chunk_win.tolist(), win_first=win_first.tolist(),
                win_last=win_last.tolist(), loc_pos=loc_pos)
    return per_core, meta


def _prep_weights(W_up_s, W_up_v, Wr1, br1, Wr2, br2, W_down_s, W_down_v,
                  Wsel_s, Wsel_v, Wsym_s, Wsym_v, Wpost_s, Wpost_v, W_read):
    f32 = lambda x: np.asarray(x, np.float32)
    w = {}
    w["wups"] = f32(W_up_s).copy()
    w["wupv"] = f32(W_up_v).copy()
    w["wr1"] = f32(Wr1).copy()
    w["br1"] = f32(br1).reshape(RH, 1).copy()
    w["wr2a"] = np.concatenate([f32(Wr2), f32(br2)[None, :]], 0).astype(BFNP)
    w["wdns"] = (f32(W_down_s) / AVG_NEIGH).copy()
    w["wdnv"] = (f32(W_down_v) / AVG_NEIGH).copy()
    w["wsels"] = f32(Wsel_s).transpose(1, 0, 2).reshape(C, S * C).copy()
    w["wselv"] = f32(Wsel_v).transpose(1, 0, 2).reshape(C, S * C).copy()
    w["wsyms"] = f32(Wsym_s).transpose(2, 0, 1).reshape(C, S * NPATH).copy()
    w["wsymv"] = f32(Wsym_v).transpose(2, 0, 1).reshape(C, S * 4).copy()
    w["wposts"] = f32(Wpost_s).copy()
    w["wpostv"] = f32(Wpost_v).copy()
    w["wread"] = f32(W_read).copy()
    w["iota"] = np.broadcast_to(np.arange(128, dtype=np.float32),
                                (128, 128)).astype(BFNP).reshape(128, 1, 128).copy()
    w["eye"] = np.eye(128, dtype=np.float32)
    return w


WDTYPES = dict(wups=F32, wupv=F32, wr1=F32, br1=F32, wr2a=BF16, wdns=F32,
               wdnv=F32, wsels=F32, wselv=F32, wsyms=F32, wsymv=F32,
               wposts=F32, wpostv=F32, wread=F32, iota=BF16, eye=F32)
WSHAPES = dict(wups=[C, C], wupv=[C, C], wr1=[RB, RH], br1=[RH, 1],
               wr2a=[RH + 1, NPATH * C], wdns=[C, C], wdnv=[C, C],
               wsels=[C, S * C], wselv=[C, S * C], wsyms=[C, S * NPATH],
               wsymv=[C, S * 4], wposts=[C, C], wpostv=[C, C], wread=[C, 1],
               iota=[128, 1, 128], eye=[128, 128])


# --------------------------------------------------------------------------
# device program
# --------------------------------------------------------------------------

def _build(meta):
    NLP, NWIN, NST = meta["NLP"], meta["NWIN"], meta["NST"]
    SEGP = meta["SEGP"]
    chunk_win = meta["chunk_win"]
    win_first, win_last = meta["win_first"], meta["win_last"]
    TROWS = CORES * NLP

    nc = bacc.Bacc("TRN2", target_bir_lowering=False, debug=False,
                   num_devices=CORES)

    din = {}
    din["vec"] = nc.dram_tensor("vec", [NST, 128, G * 3], F32, kind="ExternalInput")
    din["rad"] = nc.dram_tensor("rad", [NST, RB, G * 128], F32, kind="ExternalInput")
    din["oh"] = nc.dram_tensor("oh", [NST, 128, G], BF16, kind="ExternalInput")
    din["gx"] = nc.dram_tensor("gx", [NST, 128, 64], I16, kind="ExternalInput")
    din["nsT"] = nc.dram_tensor("nsT", [C, NLP], F32, kind="ExternalInput")
    din["nvT"] = nc.dram_tensor("nvT", [C, 3 * NLP], F32, kind="ExternalInput")
    for n, sh in WSHAPES.items():
        din[n] = nc.dram_tensor(n, sh, WDTYPES[n], kind="ExternalInput")
    out_s = nc.dram_tensor("out_s", [C, NLP], F32, kind="ExternalOutput")
    out_v = nc.dram_tensor("out_v", [C, 3 * NLP], F32, kind="ExternalOutput")
    out_r = nc.dram_tensor("out_r", [1, NLP], F32, kind="ExternalOutput")

    with tile.TileContext(nc) as tc:
        with (
            tc.tile_pool(name="cw", bufs=1) as cw,
            tc.tile_pool(name="dram", bufs=1, space="DRAM") as dram,
            tc.tile_pool(name="agg", bufs=1) as agp,
            tc.tile_pool(name="ppacc", bufs=2, space="PSUM") as ppacc,
            tc.tile_pool(name="ppmm", bufs=2, space="PSUM") as ppmm,
            tc.tile_pool(name="ppw", bufs=2, space="PSUM") as ppw,
            tc.tile_pool(name="pptr", bufs=2, space="PSUM") as pptr,
        ):
            W = {}
            for n, sh in WSHAPES.items():
                W[n] = cw.tile(sh, WDTYPES[n], tag=n, name=n)
                nc.sync.dma_start(W[n][:], din[n][:])

            tbl_slice = dram.tile([NLP, 256], BF16, tag="tsl")
            tbl_full = dram.tile([TROWS, 256], BF16, tag="tfl",
                                 addr_space="Shared")
            aggT = {p: agp.tile([C, NLP], F32, tag=f"aggT{p}", name=f"aggT{p}") for p in range(4)}
            for p in range(4):
                nc.vector.memset(aggT[p][:], 0.0)

            # ---------------- phase A: node table + AllGather ------------
            with tc.tile_pool(name="pa", bufs=1) as pa, \
                 tc.tile_pool(name="pat", bufs=2) as pat:
                nsT = pa.tile([C, NLP], F32, tag="nsT")
                nvT = pa.tile([C, 3 * NLP], F32, tag="nvT")
                nc.sync.dma_start(nsT[:], din["nsT"][:])
                nc.sync.dma_start(nvT[:], din["nvT"][:])
                for w in range(NWIN):
                    pu = ppacc.tile([128, 256], F32, tag="acc")
                    cols = slice(w * 128, (w + 1) * 128)
                    nc.tensor.matmul(out=pu[:, 0:C], lhsT=nsT[:, cols],
                                     rhs=W["wups"][:], start=True, stop=True)
                    for d in range(3):
                        nc.tensor.matmul(
                            out=pu[:, C * (1 + d):C * (2 + d)],
                            lhsT=nvT[:, d * NLP + w * 128:d * NLP + (w + 1) * 128],
                            rhs=W["wupv"][:], start=True, stop=True)
                    tb = pat.tile([128, 256], BF16, tag="tb")
                    nc.scalar.copy(tb[:], pu[:])
                    nc.sync.dma_start(tbl_slice[w * 128:(w + 1) * 128, :], tb[:])
                nc.gpsimd.collective_compute(
                    "AllGather", OP.bypass,
                    replica_groups=[list(range(CORES))],
                    ins=[tbl_slice.opt()], outs=[tbl_full.opt()])

            # ---------------- phase B: edges ------------------------------
            agg_ps = {}
            with tc.tile_pool(name="ed", bufs=2) as ed:
                for st in range(NST):
                    vec = ed.tile([128, G, 3], F32, tag="vec")
                    rad = ed.tile([RB, G * 128], F32, tag="rad")
                    oh = ed.tile([128, G], BF16, tag="oh")
                    gx = ed.tile([128, 64], I16, tag="gx")
                    nc.sync.dma_start(
                        vec[:], din["vec"][st].rearrange("p (g d) -> p g d", d=3))
                    nc.sync.dma_start(rad[:], din["rad"][st])
                    nc.sync.dma_start(oh[:], din["oh"][st])
                    nc.sync.dma_start(gx[:], din["gx"][st])

                    feat = ed.tile([128, G, 256], BF16, tag="feat")
                    nc.gpsimd.dma_gather(out_ap=feat[:], in_ap=tbl_full[:],
                                         idxs_ap=gx[:], num_idxs=G * 128,
                                         num_idxs_reg=G * 128, elem_size=256)

                    # Y1 = vec / (|vec| + 1e-9)
                    sq = ed.tile([128, G, 3], F32, tag="sq")
                    nrm = ed.tile([128, G], F32, tag="nrm")
                    rno = ed.tile([128, G], F32, tag="rno")
                    y1b = ed.tile([128, G, 3], BF16, tag="y1b")
                    nc.vector.tensor_mul(sq[:], vec[:], vec[:])
                    nc.vector.tensor_add(nrm[:], sq[:, :, 0], sq[:, :, 1])
                    nc.vector.tensor_add(nrm[:], nrm[:], sq[:, :, 2])
                    nc.scalar.sqrt(nrm[:], nrm[:])
                    nc.vector.tensor_scalar_add(nrm[:], nrm[:], 1e-9)
                    nc.vector.reciprocal(rno[:], nrm[:])
                    nc.vector.tensor_mul(sq[:], vec[:],
                                         rno[:].to_broadcast([128, G, 3]))
                    nc.vector.tensor_copy(y1b[:], sq[:])

                    # one-hot scatter matrices (bf16): O, O*Y1_d
                    Og = ed.tile([128, G, 512], BF16, tag="Og")
                    nc.vector.tensor_tensor(
                        Og[:, :, 0:128],
                        oh[:].to_broadcast([128, G, 128]),
                        W["iota"][:].to_broadcast([128, G, 128]),
                        op=OP.is_equal)
                    for d in range(3):
                        eng = nc.vector if d == 0 else nc.gpsimd
                        eng.tensor_tensor(
                            Og[:, :, 128 * (d + 1):128 * (d + 2)],
                            Og[:, :, 0:128],
                            y1b[:, :, d].to_broadcast([128, G, 128]),
                            op=OP.mult)

                    # radial MLP layer 1 (h^T) + silu into h_aug
                    haug = ed.tile([RH + 1, G * 128], BF16, tag="haug")
                    nc.vector.memset(haug[RH:RH + 1, :], 1.0)
                    sgt = ed.tile([RH, G * 128], BF16, tag="sgt")
                    for hh in range(G * 128 // 512):
                        hs = slice(hh * 512, (hh + 1) * 512)
                        ph = ppmm.tile([RH, 512], F32, tag="mm")
                        nc.tensor.matmul(out=ph[:], lhsT=W["wr1"][:],
                                         rhs=rad[:, hs], start=True, stop=True)
                        nc.scalar.activation(sgt[:, hs], ph[:], AF.Sigmoid,
                                             bias=W["br1"][:, 0:1], scale=1.0)
                        nc.scalar.activation(haug[0:RH, hs], ph[:], AF.Identity,
                                             bias=W["br1"][:, 0:1], scale=1.0)
                    nc.vector.tensor_mul(haug[0:RH, :], haug[0:RH, :], sgt[:])

                    # radial layer 2 per chunk + cast to bf16
                    wbf = ed.tile([128, G, NPATH * C], BF16, tag="wbf")
                    for j in range(G):
                        pw = ppw.tile([128, NPATH * C], F32, tag="w")
                        nc.tensor.matmul(out=pw[:],
                                         lhsT=haug[:, j * 128:(j + 1) * 128],
                                         rhs=W["wr2a"][:], start=True, stop=True)
                        nc.scalar.copy(wbf[:, j, :], pw[:])

                    # payload planes (16 slots of 64 cols)
                    upl = ed.tile([128, G, 1024], BF16, tag="upl")
                    ss = feat[:, :, 0:C]
                    sv = [feat[:, :, C * (1 + d):C * (2 + d)] for d in range(3)]
                    wp = [wbf[:, :, C * p:C * (p + 1)] for p in range(NPATH)]
                    slot = lambda i: upl[:, :, C * i:C * (i + 1)]
                    mul = nc.vector.tensor_mul
                    mul(slot(0), wp[0], ss)            # u0
                    mul(slot(1), wp[2], sv[0])         # u2_0
                    mul(slot(2), wp[2], sv[1])         # u2_1
                    mul(slot(3), wp[2], sv[2])         # u2_2
                    mul(slot(4), wp[3], sv[0])         # u3_0
                    mul(slot(5), wp[1], ss)            # u1
                    mul(slot(6), wp[4], sv[2])         # u4_2
                    mul(slot(13), wp[4], sv[1])        # u4_1
                    mul(slot(8), wp[3], sv[1])         # u3_1
                    mul(slot(11), wp[4], sv[0])        # u4_0
                    mul(slot(12), wp[3], sv[2])        # u3_2
                    nc.gpsimd.tensor_scalar_mul(slot(7), slot(13), -1.0)
                    nc.gpsimd.tensor_scalar_mul(slot(9), slot(6), -1.0)
                    nc.gpsimd.tensor_scalar_mul(slot(14), slot(11), -1.0)
                    nc.gpsimd.tensor_copy(slot(10), slot(5))
                    nc.gpsimd.tensor_copy(slot(15), slot(5))

                    # weighted one-hot scatter into window PSUM
                    for j in range(G):
                        cc = st * G + j
                        w = chunk_win[cc]
                        if win_first[cc]:
                            agg_ps[w] = ppacc.tile([128, 256], F32, tag="acc", name=f"agg{w}")
                        pt = agg_ps[w]
                        for gs in range(4):
                            nc.tensor.matmul(
                                out=pt[:],
                                lhsT=Og[:, j, 128 * gs:128 * (gs + 1)],
                                rhs=upl[:, j, 256 * gs:256 * (gs + 1)],
                                start=(win_first[cc] and gs == 0),
                                stop=(win_last[cc] and gs == 3))
                        if win_last[cc]:
                            asb = ed.tile([128, 256], F32, tag="asb")
                            nc.scalar.copy(asb[:], pt[:])
                            wcols = slice(w * 128, (w + 1) * 128)
                            for half in range(2):
                                ptr = pptr.tile([128, 128], F32, tag="tr")
                                nc.tensor.matmul(
                                    out=ptr[:],
                                    lhsT=asb[:, 128 * half:128 * (half + 1)],
                                    rhs=W["eye"][:], start=True, stop=True)
                                nc.scalar.copy(aggT[2 * half][:, wcols],
                                               ptr[0:C, :])
                                nc.scalar.copy(aggT[2 * half + 1][:, wcols],
                                               ptr[C:128, :])

            # ---------------- phase C: node pipeline ----------------------
            with tc.tile_pool(name="nd", bufs=1) as nd, \
                 tc.tile_pool(name="sg", bufs=2) as sg:
                s1 = nd.tile([C, NLP], F32, tag="t1")
                v1 = nd.tile([C, 3 * NLP], F32, tag="t2")
                for t0 in range(0, NLP, 512):
                    tl = slice(t0, min(t0 + 512, NLP))
                    n = tl.stop - tl.start
                    pd = ppmm.tile([C, 512], F32, tag="mm")
                    nc.tensor.matmul(out=pd[:, 0:n], lhsT=W["wdns"][:],
                                     rhs=aggT[0][:, tl], start=True, stop=True)
                    nc.scalar.copy(s1[:, tl], pd[:, 0:n])
                    for d in range(3):
                        pv = ppmm.tile([C, 512], F32, tag="mm")
                        nc.tensor.matmul(out=pv[:, 0:n], lhsT=W["wdnv"][:],
                                         rhs=aggT[1 + d][:, tl],
                                         start=True, stop=True)
                        nc.scalar.copy(v1[:, d * NLP + tl.start:d * NLP + tl.stop],
                                       pv[:, 0:n])

                z = nd.tile([C, NLP], F32, tag="z")
                vq = nd.tile([C, 3 * NLP], F32, tag="vq")
                stt = nc.vector.scalar_tensor_tensor
                for s in range(S):
                    seg = slice(s * SEGP, (s + 1) * SEGP)
                    # selector matmuls for this species segment
                    seg_se = sg.tile([C, SEGP], F32, tag="se")
                    seg_ve = sg.tile([C, 3 * SEGP], F32, tag="ve")
                    ps = ppw.tile([C, SEGP], F32, tag="w")
                    nc.tensor.matmul(out=ps[:],
                                     lhsT=W["wsels"][:, s * C:(s + 1) * C],
                                     rhs=s1[:, seg], start=True, stop=True)
                    nc.scalar.copy(seg_se[:], ps[:])
                    for d in range(3):
                        pv = ppw.tile([C, SEGP], F32, tag="w")
                        nc.tensor.matmul(
                            out=pv[:], lhsT=W["wselv"][:, s * C:(s + 1) * C],
                            rhs=v1[:, d * NLP + seg.start:d * NLP + seg.stop],
                            start=True, stop=True)
                        nc.scalar.copy(seg_ve[:, d * SEGP:(d + 1) * SEGP], pv[:])

                    # symmetric contraction for this segment
                    sed = [seg_ve[:, d * SEGP:(d + 1) * SEGP] for d in range(3)]
                    se2 = sg.tile([C, SEGP], F32, tag="se2")
                    vv = sg.tile([C, SEGP], F32, tag="vv")
                    tA = sg.tile([C, SEGP], F32, tag="tA")
                    tB = sg.tile([C, SEGP], F32, tag="tB")
                    cs = lambda p: W["wsyms"][:, s * NPATH + p:s * NPATH + p + 1]
                    cv = lambda p: W["wsymv"][:, s * 4 + p:s * 4 + p + 1]
                    nc.vector.tensor_mul(se2[:], seg_se[:], seg_se[:])
                    nc.vector.tensor_mul(vv[:], sed[0], sed[0])
                    nc.gpsimd.tensor_mul(tA[:], sed[1], sed[1])
                    nc.vector.tensor_add(vv[:], vv[:], tA[:])
                    nc.gpsimd.tensor_mul(tA[:], sed[2], sed[2])
                    nc.vector.tensor_add(vv[:], vv[:], tA[:])
                    # z = se*(c0 + c1*se + c2*se2) + vv*(c3 + c4*se)
                    stt(tA[:], seg_se[:], cs(1), cs(0).to_broadcast([C, SEGP]),
                        op0=OP.mult, op1=OP.add)
                    stt(tA[:], se2[:], cs(2), tA[:], op0=OP.mult, op1=OP.add)
                    nc.vector.tensor_mul(z[:, seg], seg_se[:], tA[:])
                    stt(tB[:], seg_se[:], cs(4), cs(3).to_broadcast([C, SEGP]),
                        op0=OP.mult, op1=OP.add)
                    nc.vector.tensor_mul(tB[:], vv[:], tB[:])
                    nc.vector.tensor_add(z[:, seg], z[:, seg], tB[:])
                    # q = cv0 + cv1*se + cv2*se2 + cv3*vv ; vq_d = ve_d * q
                    stt(tA[:], seg_se[:], cv(1), cv(0).to_broadcast([C, SEGP]),
                        op0=OP.mult, op1=OP.add)
                    stt(tA[:], se2[:], cv(2), tA[:], op0=OP.mult, op1=OP.add)
                    stt(tA[:], vv[:], cv(3), tA[:], op0=OP.mult, op1=OP.add)
                    for d in range(3):
                        nc.vector.tensor_mul(
                            vq[:, d * NLP + seg.start:d * NLP + seg.stop],
                            sed[d], tA[:])

                # post linear + readout
                oS = nd.tile([C, NLP], F32, tag="t1")
                oV = nd.tile([C, 3 * NLP], F32, tag="t2")
                oR = nd.tile([1, NLP], F32, tag="oR")
                for t0 in range(0, NLP, 512):
                    tl = slice(t0, min(t0 + 512, NLP))
                    n = tl.stop - tl.start
                    pp = ppmm.tile([C, 512], F32, tag="mm")
                    nc.tensor.matmul(out=pp[:, 0:n], lhsT=W["wposts"][:],
                                     rhs=z[:, tl], start=True, stop=True)
                    nc.scalar.copy(oS[:, tl], pp[:, 0:n])
                    for d in range(3):
                        pv = ppmm.tile([C, 512], F32, tag="mm")
                        nc.tensor.matmul(
                            out=pv[:, 0:n], lhsT=W["wpostv"][:],
                            rhs=vq[:, d * NLP + tl.start:d * NLP + tl.stop],
                            start=True, stop=True)
                        nc.scalar.copy(oV[:, d * NLP + tl.start:d * NLP + tl.stop],
                                       pv[:, 0:n])
                    pr = ppw.tile([1, 512], F32, tag="w")
                    nc.tensor.matmul(out=pr[:, 0:n], lhsT=W["wread"][:],
                                     rhs=oS[:, tl], start=True, stop=True)
                    nc.scalar.copy(oR[:, tl], pr[:, 0:n])

                nc.sync.dma_start(out_s[:], oS[:])
                nc.sync.dma_start(out_v[:], oV[:])
                nc.sync.dma_start(out_r[:], oR[:])

    nc.compile()
    return nc


# --------------------------------------------------------------------------
# entry point
# --------------------------------------------------------------------------

def _in_maps(per_core, weights):
    maps = []
    for k in range(CORES):
        mm = dict(per_core[k])
        mm.update(weights)
        maps.append(mm)
    return maps


def _assemble(results, meta):
    N, NLOC, NLP = meta["N"], meta["NLOC"], meta["NLP"]
    loc_pos = meta["loc_pos"]
    s_full = np.zeros((N, C), np.float32)
    v_full = np.zeros((N, C, 3), np.float32)
    r_full = np.zeros((N, 1, 1), np.float32)
    for k in range(CORES):
        sl = slice(k * NLOC, (k + 1) * NLOC)
        lp = loc_pos[sl]
        s_full[sl] = results[k]["out_s"][:, lp].T
        ov = results[k]["out_v"].reshape(C, 3, NLP)
        v_full[sl] = ov[:, :, lp].transpose(2, 0, 1)
        r_full[sl, 0, 0] = results[k]["out_r"][0, lp]
    return r_full, s_full, v_full


def prepare(inputs):
    per_core, meta = _prep(
        inputs["vectors"], inputs["node_s"], inputs["node_v"],
        inputs["radial_embedding"], inputs["node_specie"],
        inputs["senders"], inputs["receivers"])
    weights = _prep_weights(
        inputs["W_up_s"], inputs["W_up_v"], inputs["Wr1"], inputs["br1"],
        inputs["Wr2"], inputs["br2"], inputs["W_down_s"], inputs["W_down_v"],
        inputs["Wsel_s"], inputs["Wsel_v"], inputs["Wsym_s"], inputs["Wsym_v"],
        inputs["Wpost_s"], inputs["Wpost_v"], inputs["W_read"])
    return _in_maps(per_core, weights), meta


def kernel(**inputs):
    maps, meta = prepare(inputs)
    nc = _build(meta)
    res = run_bass_kernel_spmd(nc, maps, core_ids=list(range(CORES)))
    return _assemble(res.results, meta)
